# revision 1
# baseline (speedup 1.0000x reference)
"""DiffTexture bilinear sampling kernel for TRN2 (8 NeuronCores).

Strategy (data-parallel over sample points, texture replicated):
  - Each core handles N/8 = 1,048,576 points.
  - Phase 1 (per core): build a 2x2-block table B in DRAM:
      B[u, v] = [T[u,v], T[u,v+1], T[u+1,v], T[u+1,v+1]]  (12 f32 = 48B)
    for u in [0, 2046], v in [0, 2047] (col 2047 garbage, never read).
    Built with dense DMA loads + DVE strided interleave copies.
  - Phase 2: per 128-point chunk, one indirect DMA gathers each point's
    48B block (idx = u0a*2048 + v0a); bilinear blend with adjusted
    weights folds the floor/ceil edge cases into the lerp weights:
      WU = a*mu0 + (1-a)*mu1, mu_i = (row_i != u0a)  (same for WV)
      out = lerp(lerp(p00,p01,WV), lerp(p10,p11,WV), WU); tanh on ACT.

Floor is built from the DVE round-to-nearest f32->i32 cast plus a
compare fix-up (no floor ALU op on TRN2).
"""

import numpy as np

import concourse.bass as bass
import concourse.bacc as bacc
import concourse.mybir as mybir
from concourse import tile
from concourse.bass_utils import run_bass_kernel_spmd

H = 2048
W = 2048
N_FULL = 8388608
NCORES = 8
P = 128
K = 512                  # points per partition per macro-tile
TPOINTS = P * K          # 65536 points per macro-tile

f32 = mybir.dt.float32
i32 = mybir.dt.int32

ROW = W * 3              # texture row, elements (6144)
BROW = W * 12            # block-table row, elements (24576)
BROWS = H - 1            # block-table rows built (2047)


def _ap(t_ap, extra_offset, dims):
    """Build a raw AP on the same tensor as t_ap with given dims."""
    return bass.AP(t_ap.tensor, t_ap.offset + extra_offset, dims)


def build_nc(npc):
    """Build the per-core Bass program for npc points (npc % TPOINTS == 0)."""
    ntiles = npc // TPOINTS
    nc = bacc.Bacc("TRN2", target_bir_lowering=False)

    uvs = nc.dram_tensor("uvs", [npc, 2], f32, kind="ExternalInput")
    texture = nc.dram_tensor("texture", [H, W, 3], f32, kind="ExternalInput")
    out = nc.dram_tensor("out", [npc, 3], f32, kind="ExternalOutput")
    btab = nc.dram_tensor("btab", [BROWS * W, 12], f32)  # internal, 192MB

    tex_flat = texture[:].rearrange("h w c -> (h w c)")
    uvs_t = uvs[:].rearrange("(t p k) c -> t p (k c)", t=ntiles, p=P, k=K)
    out_t = out[:].rearrange("(t p k) c -> t p (k c)", t=ntiles, p=P, k=K)

    with tile.TileContext(nc) as tc:
        # ---- Phase 1: build the 2x2 block table --------------------------
        with tc.tile_pool(name="bpool", bufs=2) as bp:
            for blk in range(16):
                u0 = blk * 128
                nr = 128 if blk < 15 else 127          # rows this block
                a_t = bp.tile([P, ROW + 3], f32, tag="arow")
                a1_t = bp.tile([P, ROW + 3], f32, tag="a1row")
                # row u .. u+nr-1, each read ROW+3 elems (next row's 1st texel)
                nc.sync.dma_start(
                    out=a_t[:nr, :],
                    in_=_ap(tex_flat, u0 * ROW, [[ROW, nr], [1, ROW + 3]]),
                )
                a1_len = ROW + 3 if blk < 15 else ROW
                nc.sync.dma_start(
                    out=a1_t[:nr, :a1_len],
                    in_=_ap(tex_flat, (u0 + 1) * ROW, [[ROW, nr], [1, a1_len]]),
                )
                for c in range(2):      # two 1024-column chunks
                    bt = bp.tile([P, 12 * 1024], f32, tag="bchunk")
                    voff = c * 1024 * 3
                    for (dst_off, src, src_off) in (
                        (0, a_t, 0), (3, a_t, 3), (6, a1_t, 0), (9, a1_t, 3),
                    ):
                        nc.vector.tensor_copy(
                            _ap(bt[:], dst_off, [bt[:].ap[0], [12, 1024], [1, 3]]),
                            _ap(src[:], voff + src_off,
                                [src[:].ap[0], [3, 1024], [1, 3]]),
                        )
                    nc.sync.dma_start(
                        out=_ap(btab[:], u0 * BROW + c * 12 * 1024,
                                [[BROW, nr], [1, 12 * 1024]]),
                        in_=bt[:nr, :],
                    )

        tc.strict_bb_all_engine_barrier()

        # ---- Phase 2: per-tile sample ------------------------------------
        with tc.tile_pool(name="main", bufs=2) as mp:
            for ti in range(ntiles):
                uv = mp.tile([P, 2 * K], f32, tag="uv")
                nc.sync.dma_start(out=uv[:], in_=uvs_t[ti])
                x_ap = _ap(uv[:], 0, [uv[:].ap[0], [2, K]])
                y_ap = _ap(uv[:], 1, [uv[:].ap[0], [2, K]])

                def coord(src_ap, name):
                    # returns (low-col weight toward high cell, clamped low idx)
                    cu = mp.tile([P, K], f32, tag=f"{name}_cu")
                    nc.vector.tensor_scalar(
                        out=cu[:], in0=src_ap, scalar1=1.0, scalar2=0.5,
                        op0=mybir.AluOpType.add, op1=mybir.AluOpType.mult)
                    nc.vector.tensor_scalar(
                        out=cu[:], in0=cu[:], scalar1=float(W - 1),
                        scalar2=None, op0=mybir.AluOpType.mult)
                    ci = mp.tile([P, K], i32, tag=f"{name}_ci")
                    nc.vector.tensor_copy(ci[:], cu[:])
                    t1 = mp.tile([P, K], f32, tag=f"{name}_t1")
                    nc.vector.tensor_copy(t1[:], ci[:])        # rcf = rint(u)
                    t2 = mp.tile([P, K], f32, tag=f"{name}_t2")
                    nc.vector.tensor_tensor(                    # t2 = rcf - u
                        out=t2[:], in0=t1[:], in1=cu[:],
                        op=mybir.AluOpType.subtract)
                    nc.vector.tensor_scalar(                    # delta=(rcf>u)
                        out=t2[:], in0=t2[:], scalar1=0.0, scalar2=0.0,
                        op0=mybir.AluOpType.max, op1=mybir.AluOpType.not_equal)
                    nc.vector.tensor_tensor(                    # t1 = i0f
                        out=t1[:], in0=t1[:], in1=t2[:],
                        op=mybir.AluOpType.subtract)
                    fr = mp.tile([P, K], f32, tag=f"{name}_fr")
                    nc.vector.tensor_tensor(                    # fr = u - i0f
                        out=fr[:], in0=cu[:], in1=t1[:],
                        op=mybir.AluOpType.subtract)
                    nc.vector.tensor_tensor(                    # t2 = (u!=i0f)
                        out=t2[:], in0=cu[:], in1=t1[:],
                        op=mybir.AluOpType.not_equal)
                    nc.vector.tensor_tensor(                    # cu = i1f
                        out=cu[:], in0=t1[:], in1=t2[:],
                        op=mybir.AluOpType.add)
                    i0af = mp.tile([P, K], f32, tag=f"{name}_i0af")
                    nc.vector.tensor_scalar(                    # clamp
                        out=i0af[:], in0=t1[:], scalar1=float(W - 2),
                        scalar2=None, op0=mybir.AluOpType.min)
                    nc.vector.tensor_tensor(                    # t1 = m0
                        out=t1[:], in0=t1[:], in1=i0af[:],
                        op=mybir.AluOpType.not_equal)
                    nc.vector.tensor_tensor(                    # t2 = m1
                        out=t2[:], in0=cu[:], in1=i0af[:],
                        op=mybir.AluOpType.not_equal)
                    # wt = m1 + fr*(m0-m1)
                    nc.vector.tensor_tensor(
                        out=t1[:], in0=t1[:], in1=t2[:],
                        op=mybir.AluOpType.subtract)
                    nc.vector.tensor_tensor(
                        out=t1[:], in0=t1[:], in1=fr[:],
                        op=mybir.AluOpType.mult)
                    wt = mp.tile([P, K], f32, tag=f"{name}_wt")
                    nc.vector.tensor_tensor(
                        out=wt[:], in0=t1[:], in1=t2[:],
                        op=mybir.AluOpType.add)
                    return wt, i0af

                wu, u0af = coord(x_ap, "u")
                wv, v0af = coord(y_ap, "v")

                # idx = u0af*2048 + v0af -> int32
                idxf = mp.tile([P, K], f32, tag="idxf")
                nc.vector.scalar_tensor_tensor(
                    out=idxf[:], in0=u0af[:], scalar=float(W), in1=v0af[:],
                    op0=mybir.AluOpType.mult, op1=mybir.AluOpType.add)
                idx = mp.tile([P, K], i32, tag="idx")
                nc.vector.tensor_copy(idx[:], idxf[:])

                # gather 48B blocks
                patch = mp.tile([P, 12 * K], f32, tag="patch")
                for k in range(K):
                    nc.gpsimd.indirect_dma_start(
                        out=patch[:, 12 * k:12 * (k + 1)],
                        out_offset=None,
                        in_=btab[:],
                        in_offset=bass.IndirectOffsetOnAxis(
                            ap=idx[:, k:k + 1], axis=0),
                    )

                # blend
                pap = patch[:]
                p00 = _ap(pap, 0, [pap.ap[0], [12, K], [1, 3]])
                p01 = _ap(pap, 3, [pap.ap[0], [12, K], [1, 3]])
                p10 = _ap(pap, 6, [pap.ap[0], [12, K], [1, 3]])
                p11 = _ap(pap, 9, [pap.ap[0], [12, K], [1, 3]])
                wv3 = mp.tile([P, 3 * K], f32, tag="wv3")
                wu3 = mp.tile([P, 3 * K], f32, tag="wu3")
                for ch in range(3):
                    nc.vector.tensor_copy(
                        _ap(wv3[:], ch, [wv3[:].ap[0], [3, K], [1, 1]]),
                        _ap(wv[:], 0, [wv[:].ap[0], [1, K], [1, 1]]))
                    nc.vector.tensor_copy(
                        _ap(wu3[:], ch, [wu3[:].ap[0], [3, K], [1, 1]]),
                        _ap(wu[:], 0, [wu[:].ap[0], [1, K], [1, 1]]))
                wvb = _ap(wv3[:], 0, [wv3[:].ap[0], [3, K], [1, 3]])
                wub = _ap(wu3[:], 0, [wu3[:].ap[0], [3, K], [1, 3]])

                def v3(t):
                    return _ap(t[:], 0, [t[:].ap[0], [3, K], [1, 3]])

                r0 = mp.tile([P, 3 * K], f32, tag="r0")
                r1 = mp.tile([P, 3 * K], f32, tag="r1")
                res = mp.tile([P, 3 * K], f32, tag="res")
                # r0 = p00 + WV*(p01-p00)
                nc.vector.tensor_tensor(out=v3(r0), in0=p01, in1=p00,
                                        op=mybir.AluOpType.subtract)
                nc.vector.tensor_tensor(out=v3(r0), in0=v3(r0), in1=wvb,
                                        op=mybir.AluOpType.mult)
                nc.vector.tensor_tensor(out=v3(r0), in0=v3(r0), in1=p00,
                                        op=mybir.AluOpType.add)
                # r1 = p10 + WV*(p11-p10)
                nc.vector.tensor_tensor(out=v3(r1), in0=p11, in1=p10,
                                        op=mybir.AluOpType.subtract)
                nc.vector.tensor_tensor(out=v3(r1), in0=v3(r1), in1=wvb,
                                        op=mybir.AluOpType.mult)
                nc.vector.tensor_tensor(out=v3(r1), in0=v3(r1), in1=p10,
                                        op=mybir.AluOpType.add)
                # res = r0 + WU*(r1-r0)   (WU = weight of the +1 row)
                nc.vector.tensor_tensor(out=v3(res), in0=v3(r1), in1=v3(r0),
                                        op=mybir.AluOpType.subtract)
                nc.vector.tensor_tensor(out=v3(res), in0=v3(res), in1=wub,
                                        op=mybir.AluOpType.mult)
                nc.vector.tensor_tensor(out=v3(res), in0=v3(res), in1=v3(r0),
                                        op=mybir.AluOpType.add)
                # tanh + store
                nc.scalar.activation(out=res[:], in_=res[:],
                                     func=mybir.ActivationFunctionType.Tanh)
                nc.sync.dma_start(out=out_t[ti], in_=res[:])

    nc.compile()
    return nc


_NC_CACHE = {}


def _get_nc(npc):
    if npc not in _NC_CACHE:
        _NC_CACHE[npc] = build_nc(npc)
    return _NC_CACHE[npc]


def kernel(uvs, texture):
    uvs = np.ascontiguousarray(uvs, dtype=np.float32)
    texture = np.ascontiguousarray(texture, dtype=np.float32)
    assert uvs.shape == (N_FULL, 2) and texture.shape == (H, W, 3)
    npc = N_FULL // NCORES
    nc = _get_nc(npc)
    in_maps = [
        {"uvs": uvs[c * npc:(c + 1) * npc], "texture": texture}
        for c in range(NCORES)
    ]
    res = run_bass_kernel_spmd(nc, in_maps, core_ids=list(range(NCORES)))
    return np.concatenate([r["out"] for r in res.results], axis=0)

